# revision 24
# baseline (speedup 1.0000x reference)
"""Trainium2 Bass kernel for DY_Block (EfficientAT DyMN dynamic block).

Data-parallel over batch B=32 across 8 NeuronCores (4 samples/core).

Key techniques vs the naive formulation:
  - x loaded ONCE per sample via a gpsimd casting DMA (f32 DRAM -> bf16 SBUF);
    output stored via casting DMA (bf16 SBUF -> f32 DRAM).
  - All weight matmuls in bf16; biases ride matmuls as extra contraction rows
    (ones-row in the rhs), which is free since PE cost depends only on the
    output free size.
  - Expansion BN+hswish fused into ONE custom-DVE op (HSWISH_PSUM_ANT)
    that reads PSUM and writes the zero-padded fp8 depthwise input layout.
  - Depthwise 3x3 as fp8 DoubleRow matmuls: per-channel weights on diagonal
    lhsT tiles, taps paired along df (pair stride 128, ISA-aligned); odd
    dt-shifts are absorbed by shifting the PSUM write window instead of the
    (2B-aligned) read window; 512-col matmul halves respect the one-bank
    PSUM output limit while DVE/ACT epilogues read the full 1024-col tile.
  - DyReLU as y1/y2 on ACT (per-channel scale+bias cols) + max on DVE (2x),
    CoordAtt gating split as sigma_f on GPSIMD + sigma_t on DVE (2x).
  - Projection accumulates the residual via an identity matmul on bf16 x;
    output bias rides the ACT eviction.
"""
import dataclasses
import os
import zlib

import ml_dtypes
import numpy as np

import concourse.bass as bass
import concourse.bacc as bacc
import concourse.tile as tile
from concourse import mybir
from concourse.bass_utils import run_bass_kernel_spmd

import concourse.dve_ops as _dvo
from concourse.dve_spec import (
    Spec, Src0, C0, C1, C2, One, relu, minn,
    lower as _dve_lower, _has_src1,
)
from concourse.dve_uop import DveOpSpec

F32 = mybir.dt.float32
BF16 = mybir.dt.bfloat16
FP8 = mybir.dt.float8e4
AX = mybir.AxisListType
OP = mybir.AluOpType
AF = mybir.ActivationFunctionType
DRM = mybir.MatmulPerfMode.DoubleRow

B, CIN, CEXP, COUT, F, T = 32, 80, 480, 80, 32, 125
CTX, K, M = 120, 4, 2
TEMP = 30.0
EPS = 1e-3
NCORES = 8
S = B // NCORES
NBLK = CEXP // CTX
FT = F * T               # 4000
TP = 128                 # padded row stride in the fp8 depthwise layout
NR = 35                  # padded rows: f=-1..32 -> rows 0..33; row 34 zero
TOFF = 2                 # t offset inside a padded row
NCH = 4                  # 1000-col chunks per [*, 4000] plane
CHW = FT // NCH          # 1000
GFR = 8                  # f-rows per dep psum tile ([*, 8*128=1024] = 2 banks)

_VTAG = (zlib.crc32(open(__file__, 'rb').read()) % 997) + 2


def _register_dve_op(name, spec):
    for o in _dvo.OPS:
        if o.name == name:
            return o
    opcode = _dvo._CUSTOM_DVE_ROW_BASE + len(_dvo.OPS)
    shas = {}
    for ver in ("v3", "v4"):
        try:
            u = _dve_lower(spec, ver=ver)
            shas[ver] = DveOpSpec(
                name=name, opcode=opcode, uops=u, rd1_en=_has_src1(spec)
            ).sha(ver)
        except Exception:
            pass
    op = _dvo.DveOp(name, spec, subdim=False, uops_sha=shas)
    _dvo.OPS.append(op)
    _dvo._SUB_OPCODE_FOR_NAME[name] = opcode
    _dvo.CUSTOM_DVE_SPECS[name] = spec
    return op


# out = (psum + s1) * clip(psum*imm2 + s0, 0, 1); with imm2=1/6, s0=0.5,
# s1=0 this is exactly hswish(psum) (bias pre-added into psum by a ones-row).
HSWISH_PSUM = _register_dve_op(
    "HSWISH_PSUM_ANT",
    Spec(
        body=(Src0 + C1) * minn(relu(Src0 * C2 + C0), One),
        reference=lambda in0, in1, s0, s1, imm2: (
            (in0.astype(np.float32) + s1)
            * np.minimum(np.maximum(in0.astype(np.float32) * imm2 + s0, 0.0), 1.0)
        ),
    ),
)

# 12 lhsT diag slots: tap order; None = zero slot (DR pair filler)
_SLOTS = [(-1, -1), (0, -1), (-1, 0), (0, 0), (-1, 1), (0, 1),
          (1, -1), None, (1, 0), None, (1, 1), None]


def _ap_with(v, dims, offset):
    return dataclasses.replace(
        v, ap=type(v.ap)([list(v.ap[0])] + dims), offset=offset)


def _emit(tc, io, ctx):
    nc = tc.nc

    (xs, jlf, jlt, cvf, cvt, bfb, btb, arw, drw, expw, ebrow,
     depw, bnbd, projw, pbias, identf, vtag, identb, onesbf, yout) = io

    wpool = ctx.enter_context(tc.tile_pool(name="weights", bufs=1))
    ctx_pool = ctx.enter_context(tc.tile_pool(name="ctx", bufs=1))
    pspool = ctx.enter_context(tc.tile_pool(name="ps", bufs=2, space="PSUM"))
    work = ctx.enter_context(tc.tile_pool(name="work", bufs=2))
    xpool = ctx.enter_context(tc.tile_pool(name="xpool", bufs=1))
    zpool = ctx.enter_context(tc.tile_pool(name="zpool", bufs=1))

    def wtile(ap, tag):
        t = wpool.tile(list(ap.shape), ap.dtype, tag=tag)
        nc.sync.dma_start(t[:], ap)
        return t

    w_jlf = wtile(jlf, "jlf")          # [81,120] bf16
    w_jlt = wtile(jlt, "jlt")          # [81,120] bf16
    w_cvf = wtile(cvf, "cvf")          # [120,480] bf16
    w_cvt = wtile(cvt, "cvt")          # [120,480] bf16
    w_bfb = wtile(bfb, "bfb")          # [120,4] f32
    w_btb = wtile(btb, "btb")          # [120,4] f32
    w_arw = wtile(arw, "arw")          # [121,12] bf16
    w_drw = wtile(drw, "drw")          # [121,1920] bf16
    w_expw = wtile(expw, "expw")       # [80,1920] bf16 (k-major)
    w_ebrow = wtile(ebrow, "ebrow")    # [1,480] bf16
    w_depw = wtile(depw, "depw")       # [120, K*48] bf16 (k-major, 12 slots/blk)
    w_bnbd = wtile(bnbd, "bnbd")       # [120,4] f32
    w_projw = wtile(projw, "projw")    # [120, K*320] bf16 (k-major)
    w_pbias = wtile(pbias, "pbias")    # [80,1] f32
    w_idf = wtile(identf, "idf")       # [128,128] f32
    w_idb = wtile(identb, "idb")       # [128,128] bf16

    # ---------- load x (cast f32->bf16), ones rows ----------
    x0bs = []
    for s in range(S):
        xb = xpool.tile([CIN + 1, FT], BF16, tag=f"x0b{s}")
        nc.gpsimd.dma_start(xb[0:CIN, :], xs[s])
        nc.sync.dma_start(xb[CIN:CIN + 1, :], onesbf[0:1, 0:FT])
        x0bs.append(xb)

    # ---------- Phase A : context ----------
    ga = ctx_pool.tile([CIN + 1, S * (F + T)], BF16)
    nc.sync.dma_start(ga[CIN:CIN + 1, :], onesbf[0:1, 0:S * (F + T)])
    for s in range(S):
        nc.vector.tensor_reduce(
            ga[0:CIN, s * F:(s + 1) * F],
            x0bs[s][0:CIN].rearrange("p (f t) -> p f t", t=T), AX.X, OP.add)
        h1 = work.tile([CIN, 2000], BF16, tag="cth1")
        nc.vector.tensor_add(h1[:], x0bs[s][0:CIN, 0:2000], x0bs[s][0:CIN, 2000:4000])
        h2 = work.tile([CIN, 1000], BF16, tag="cth2")
        nc.vector.tensor_add(h2[:], h1[:, 0:1000], h1[:, 1000:2000])
        h3 = work.tile([CIN, 500], BF16, tag="cth3")
        nc.vector.tensor_add(h3[:], h2[:, 0:500], h2[:, 500:1000])
        h4 = work.tile([CIN, 250], BF16, tag="cth4")
        nc.vector.tensor_add(h4[:], h3[:, 0:250], h3[:, 250:500])
        nc.vector.tensor_add(ga[0:CIN, S * F + s * T: S * F + (s + 1) * T],
                             h4[:, 0:125], h4[:, 125:250])

    # joint conv + BN + hswish: psum has bias via ones-row
    SF, ST = S * F, S * T
    ps1 = pspool.tile([128, 1024], F32, tag="pse")
    nc.tensor.matmul(ps1[0:CTX, 0:SF], w_jlf[:], ga[:, 0:SF], start=True, stop=True)
    nc.tensor.matmul(ps1[0:CTX, 512:512 + ST], w_jlt[:], ga[:, SF:],
                     start=True, stop=True)
    gc_t = ctx_pool.tile([CTX, S * (F + T)], BF16)
    nc.vector._custom_dve(HSWISH_PSUM, out=gc_t[:, 0:SF], in0=ps1[0:CTX, 0:SF],
                          s0=0.5, s1=0.0, imm2=1.0 / 6.0)
    nc.vector._custom_dve(HSWISH_PSUM, out=gc_t[:, SF:], in0=ps1[0:CTX, 512:512 + ST],
                          s0=0.5, s1=0.0, imm2=1.0 / 6.0)

    g_c = ctx_pool.tile([CTX + 1, S], F32)
    tmp_r = ctx_pool.tile([CTX, S], F32)
    nc.vector.tensor_reduce(
        g_c[0:CTX, :], gc_t[:, 0:SF].rearrange("p (s f) -> p s f", s=S),
        AX.X, OP.add)
    nc.vector.tensor_reduce(
        tmp_r[:], gc_t[:, SF:].rearrange("p (s t) -> p s t", s=S), AX.X, OP.add)
    nc.vector.tensor_add(g_c[0:CTX, :], g_c[0:CTX, :], tmp_r[:])
    nc.gpsimd.dma_start(g_c[CTX:CTX + 1, :], onesbf[0:1, 0:S])  # cast bf16->f32
    g_cb = ctx_pool.tile([CTX + 1, S], BF16)
    nc.vector.tensor_copy(g_cb[:], g_c[:])

    # routing attention (3 heads x K experts)
    ps_a = pspool.tile([128, 1024], F32, tag="pse")
    nc.tensor.matmul(ps_a[0:S, 0:3 * K], g_cb[:], w_arw[:], start=True, stop=True)
    ex_t = ctx_pool.tile([S, 3 * K], F32)
    nc.scalar.activation(ex_t[:], ps_a[0:S, 0:3 * K], AF.Exp)
    s3 = ctx_pool.tile([S, 3], F32)
    nc.vector.tensor_reduce(
        s3[:], ex_t.rearrange("p (j k) -> p j k", j=3), AX.X, OP.add)
    rec3 = ctx_pool.tile([S, 3], F32)
    nc.vector.reciprocal(rec3[:], s3[:])
    attn = ctx_pool.tile([S, 3 * K], F32)
    for j in range(3):
        nc.vector.tensor_scalar(
            attn[:, j * K:(j + 1) * K], ex_t[:, j * K:(j + 1) * K],
            rec3[:, j:j + 1], None, OP.mult)
    att48 = ctx_pool.tile([S, 3 * K * S], F32)
    nc.vector.tensor_tensor(
        att48.rearrange("p (jk s) -> p jk s", s=S),
        attn.unsqueeze(2).broadcast_to((S, 3 * K, S)),
        w_idf[0:S, 0:S].unsqueeze(1).broadcast_to((S, 3 * K, S)),
        OP.mult)
    onesS = ctx_pool.tile([S, CTX], F32)
    nc.vector.memset(onesS[:], 1.0)
    ps_ab = pspool.tile([128, 1024], F32, tag="pse")
    nc.tensor.matmul(ps_ab[0:CTX, 0:3 * K * S], onesS[:], att48[:],
                     start=True, stop=True)
    att_b = ctx_pool.tile([CTX, 3 * K * S], F32)
    nc.scalar.copy(att_b[:], ps_ab[0:CTX, 0:3 * K * S])

    # DyReLU coefficients -> cj[j] [CTX, NBLK*S] (a1, a2, b1', b2')
    coefs = ctx_pool.tile([S, 2 * M * CEXP], F32)
    for j in range(4):
        ps_th = pspool.tile([128, 1024], F32, tag="pse")
        nc.tensor.matmul(ps_th[0:S, 0:CEXP], g_cb[:],
                         w_drw[:, j * CEXP:(j + 1) * CEXP], start=True, stop=True)
        nc.scalar.activation(coefs[:, j * CEXP:(j + 1) * CEXP],
                             ps_th[0:S, 0:CEXP], AF.Sigmoid)
    for j, (sc, of) in enumerate([(2.0, 0.0), (2.0, -1.0), (1.0, -0.5), (1.0, -0.5)]):
        nc.vector.tensor_scalar(coefs[:, j * CEXP:(j + 1) * CEXP],
                                coefs[:, j * CEXP:(j + 1) * CEXP],
                                sc, of, OP.mult, OP.add)
    cj = []
    for j in range(4):
        cj_t = ctx_pool.tile([CTX, NBLK * S], F32, tag=f"cj{j}")
        for blk in range(NBLK):
            ps_c = pspool.tile([128, 1024], F32, tag="pse")
            nc.tensor.transpose(
                ps_c[0:CTX, 0:S],
                coefs[:, j * CEXP + blk * CTX: j * CEXP + (blk + 1) * CTX],
                w_idf[0:S, 0:S])
            nc.scalar.copy(cj_t[:, blk * S:(blk + 1) * S], ps_c[0:CTX, 0:S])
        cj.append(cj_t)
    for i in range(2):  # fold dep-BN bias: b'_i = a_i*bnb + b_i
        for blk in range(NBLK):
            sl = slice(blk * S, (blk + 1) * S)
            nc.vector.scalar_tensor_tensor(
                cj[2 + i][:, sl], cj[i][:, sl], w_bnbd[:, blk:blk + 1],
                cj[2 + i][:, sl], OP.mult, OP.add)

    # CoordAtt gates (bf16)
    sigf = ctx_pool.tile([CTX, NBLK * S * F], BF16)
    sigt = ctx_pool.tile([CTX, NBLK * S * T], BF16)
    for blk in range(NBLK):
        ps_f = pspool.tile([128, 1024], F32, tag="pse")
        nc.tensor.matmul(ps_f[0:CTX, 0:SF], w_cvf[:, blk * CTX:(blk + 1) * CTX],
                         gc_t[:, 0:SF], start=True, stop=True)
        nc.scalar.activation(sigf[:, blk * SF:(blk + 1) * SF], ps_f[0:CTX, 0:SF],
                             AF.Sigmoid, bias=w_bfb[:, blk:blk + 1], scale=1.0)
        ps_t2 = pspool.tile([128, 1024], F32, tag="pse")
        nc.tensor.matmul(ps_t2[0:CTX, 0:ST], w_cvt[:, blk * CTX:(blk + 1) * CTX],
                         gc_t[:, SF:], start=True, stop=True)
        nc.scalar.activation(sigt[:, blk * ST:(blk + 1) * ST], ps_t2[0:CTX, 0:ST],
                             AF.Sigmoid, bias=w_btb[:, blk:blk + 1], scale=1.0)

    # ---------- Phase B : per-sample pipeline ----------
    sb_it = 0

    def make_mixes(s):
        def acol(jr, k):
            c0 = (jr * K + k) * S + s
            return att_b[:, c0:c0 + 1]

        # expert-weight mixes (tensor_scalar 4x + tensor_tensor 2x trees)
        def mix(dst, parts, src, w, jr, tg):
            t0 = work.tile([parts, w], BF16, tag=f"mx0{tg}", name=f"mx0{tg}{s}")
            t1 = work.tile([parts, w], BF16, tag=f"mx1{tg}", name=f"mx1{tg}{s}")
            nc.vector.tensor_scalar(t0[:], src(0), acol(jr, 0)[0:parts], None, OP.mult)
            nc.vector.tensor_scalar(t1[:], src(1), acol(jr, 1)[0:parts], None, OP.mult)
            nc.vector.tensor_tensor(t0[:], t0[:], t1[:], OP.add)
            nc.vector.tensor_scalar(t1[:], src(2), acol(jr, 2)[0:parts], None, OP.mult)
            nc.vector.tensor_tensor(t0[:], t0[:], t1[:], OP.add)
            nc.vector.tensor_scalar(t1[:], src(3), acol(jr, 3)[0:parts], None, OP.mult)
            nc.vector.tensor_tensor(dst, t0[:], t1[:], OP.add)

        web = work.tile([CIN + 1, CEXP], BF16, tag="web", name=f"web{s}")
        mix(web[0:CIN, :], CIN, lambda k: w_expw[:, k * CEXP:(k + 1) * CEXP],
            CEXP, 0, "e")
        nc.sync.dma_start(web[CIN:CIN + 1, :], ebrow)

        wd = work.tile([CTX, NBLK * 12], BF16, tag="wd", name=f"wd{s}")
        mix(wd[:], CTX, lambda k: w_depw[:, k * 48:(k + 1) * 48], NBLK * 12, 1, "d")

        wpb = work.tile([CTX, NBLK * COUT], BF16, tag="wpb", name=f"wpb{s}")
        mix(wpb[:], CTX, lambda k: w_projw[:, k * NBLK * COUT:(k + 1) * NBLK * COUT],
            NBLK * COUT, 2, "p")
        return web, wd, wpb

    mixes_next = make_mixes(0)
    for s in range(S):
        web, wd, wpb = mixes_next
        yo = work.tile([COUT, FT], F32, tag="yo")
        zs = [zpool.tile([CTX, FT], BF16, tag=f"z{blk}", bufs=2, name=f"z{blk}_{s}")
              for blk in range(NBLK)]

        # Software pipeline over the 16 (blk, G) units with a 2-stage lag so
        # each engine's in-order queue always has ready work:
        #   step i emits: matmuls(unit i) | y-stage(unit i-1) | gate(unit i-2)
        blk_state = {}

        def exp_chunk(blk, c, xe3):
            pse_t = pspool.tile([128, 1024], F32, tag="pse")
            lhs = web[:, blk * CTX:(blk + 1) * CTX]
            nc.tensor.matmul(pse_t[0:CTX, 0:512], lhs,
                             x0bs[s][:, c * CHW:c * CHW + 512],
                             start=True, stop=True)
            nc.tensor.matmul(pse_t[0:CTX, 512:1000], lhs,
                             x0bs[s][:, c * CHW + 512:(c + 1) * CHW],
                             start=True, stop=True)
            nc.vector._custom_dve(
                HSWISH_PSUM,
                out=xe3[:, 1 + GFR * c:1 + GFR * (c + 1), TOFF:TOFF + T],
                in0=pse_t[0:CTX, 0:CHW].rearrange("p (f t) -> p f t", t=T),
                s0=0.5, s1=0.0, imm2=1.0 / 6.0)

        def stage_exp(i):
            # prefetch: exp chunk (blk, c) lands 2 steps before dep unit
            # (blk, c-1) consumes it; exactly one chunk per step.
            nonlocal sb_it
            blk, c = divmod(i + 2, NCH)
            if blk >= NBLK:
                return
            if c == 0:
                dg = work.tile([CTX, 12 * TP], FP8, tag="dg")
                nc.gpsimd.tensor_tensor(
                    dg[:, 0:7 * TP].rearrange("p (i c) -> p i c", i=7),
                    w_idb[0:CTX, 0:TP].unsqueeze(1).broadcast_to((CTX, 7, TP)),
                    wd[:, blk * 12:blk * 12 + 7]
                    .unsqueeze(2).broadcast_to((CTX, 7, TP)),
                    OP.mult)
                nc.vector.tensor_tensor(
                    dg[:, 7 * TP:].rearrange("p (i c) -> p i c", i=5),
                    w_idb[0:CTX, 0:TP].unsqueeze(1).broadcast_to((CTX, 5, TP)),
                    wd[:, blk * 12 + 7:(blk + 1) * 12]
                    .unsqueeze(2).broadcast_to((CTX, 5, TP)),
                    OP.mult)
                xe = work.tile([CTX, NR * TP], FP8, tag="xe")
                xe3 = xe[:].rearrange("p (f t) -> p f t", t=TP)
                if sb_it < 2:
                    nc.vector.memset(xe[:, 0:TP], 0.0)
                    nc.vector.memset(xe[:, 33 * TP:NR * TP], 0.0)
                    nc.vector.memset(xe3[:, 1:33, 0:TOFF], 0.0)
                    nc.vector.memset(xe3[:, 1:33, TOFF + T:TP], 0.0)
                sb_it += 1
                blk_state[blk] = (dg, xe, xe3)
            exp_chunk(blk, c, blk_state[blk][2])

        def stage_mm(i):
            blk, G = divmod(i, NCH)
            dg, xe, xe3 = blk_state[blk]
            psd_t = pspool.tile([128, 1024], F32, tag="psd")
            f0 = GFR * G
            for h in range(2):
                CB0 = (f0 + 4 * h) * TP
                CB2 = (f0 + 4 * h + 2) * TP
                plan = [(0, CB0, 1, 510), (1, CB0, 0, 512), (2, CB0 + 2, 1, 510),
                        (3, CB2, 1, 510), (4, CB2, 0, 512), (5, CB2 + 2, 1, 510)]
                for k, (p, W, o, L) in enumerate(plan):
                    nc.tensor.matmul(
                        psd_t[0:CTX, 512 * h + o: 512 * h + o + L],
                        _ap_with(dg[:], [[TP, 2], [1, CTX]], 2 * p * TP),
                        _ap_with(xe[:], [[TP, 2], [1, L]], W),
                        start=(k == 0), stop=(k == 5),
                        perf_mode=DRM, skip_group_check=True)
            return psd_t

        def stage_y(i, psd_t):
            blk, G = divmod(i, NCH)
            d3 = psd_t[0:CTX].rearrange("p (f t) -> p f t", t=TP)[:, 0:GFR,
                                                                 TOFF:TOFF + T]
            col = blk * S + s
            y1 = work.tile([CTX, CHW], BF16, tag="y1")
            nc.scalar.activation(y1[:].rearrange("p (f t) -> p f t", t=T), d3,
                                 AF.Identity, bias=cj[2][:, col:col + 1],
                                 scale=cj[0][:, col:col + 1])
            y2 = work.tile([CTX, CHW], BF16, tag="y2")
            nc.scalar.activation(y2[:].rearrange("p (f t) -> p f t", t=T), d3,
                                 AF.Identity, bias=cj[3][:, col:col + 1],
                                 scale=cj[1][:, col:col + 1])
            m = work.tile([CTX, CHW], BF16, tag="m", bufs=2)
            nc.vector.tensor_tensor(m[:], y1[:], y2[:], OP.max)
            return m

        def stage_gate_f(i, m):
            blk, G = divmod(i, NCH)
            col = blk * S + s
            m3 = m[:].rearrange("p (f t) -> p f t", t=T)
            gf_v = sigf[:, col * F + GFR * G: col * F + GFR * (G + 1)] \
                .unsqueeze(2).broadcast_to((CTX, GFR, T))
            nc.gpsimd.tensor_tensor(m3, m3, gf_v, OP.mult)

        def stage_gate_t(i, m):
            blk, G = divmod(i, NCH)
            col = blk * S + s
            m3 = m[:].rearrange("p (f t) -> p f t", t=T)
            gt_v = sigt[:, col * T:(col + 1) * T] \
                .unsqueeze(1).broadcast_to((CTX, GFR, T))
            nc.vector.tensor_tensor(
                zs[blk][:, G * CHW:(G + 1) * CHW].rearrange("p (f t) -> p f t", t=T),
                m3, gt_v, OP.mult)

        def proj_chunk(c):
            psp = pspool.tile([128, 1024], F32, tag="pse")
            for lo, ln in ((0, 512), (512, 488)):
                for blk in range(NBLK):
                    nc.tensor.matmul(
                        psp[0:COUT, lo:lo + ln],
                        wpb[:, blk * COUT:(blk + 1) * COUT],
                        zs[blk][:, c * CHW + lo:c * CHW + lo + ln],
                        start=(blk == 0), stop=False, skip_group_check=True)
                nc.tensor.matmul(
                    psp[0:COUT, lo:lo + ln], w_idb[0:CIN, 0:COUT],
                    x0bs[s][0:CIN, c * CHW + lo:c * CHW + lo + ln],
                    start=False, stop=True, skip_group_check=True)
            nc.scalar.activation(yo[:, c * CHW:(c + 1) * CHW], psp[0:COUT, 0:CHW],
                                 AF.Identity, bias=w_pbias[:, 0:1], scale=1.0)

        # step i emits: exp(i+2) | matmuls(i) | y-stage(i-1) | gates(i-2)
        NU = NBLK * NCH
        fifo = {}
        for i in range(-2, NU + 2):
            stage_exp(i)
            if 0 <= i < NU:
                fifo[i] = (stage_mm(i),)
            if 0 <= i - 1 < NU:
                fifo[i - 1] = (stage_y(i - 1, fifo[i - 1][0]),)
            if 0 <= i - 2 < NU:
                stage_gate_f(i - 2, fifo[i - 2][0])
                stage_gate_t(i - 2, fifo.pop(i - 2)[0])
                # proj chunk c only needs z cols c*CHW of every block; the
                # last writer of those is unit (3, c) = index 12+c.
                if i - 2 == NU - NCH and s + 1 < S:
                    mixes_next = make_mixes(s + 1)
                if i - 2 >= NU - NCH:
                    proj_chunk(i - 2 - (NU - NCH))
        nc.sync.dma_start(yout[s], yo[:])


def _host_prep(inputs):
    p = {k: np.asarray(v, dtype=np.float32) for k, v in inputs.items()}
    bf = ml_dtypes.bfloat16

    inv_j = p["cg_joint_gamma"] / np.sqrt(p["cg_joint_var"] + EPS)
    sh_j = p["cg_joint_beta"] - p["cg_joint_mean"] * inv_j
    jlf = np.vstack([(p["cg_joint_w"].T * inv_j[None, :]) / T, sh_j[None, :]])
    jlt = np.vstack([(p["cg_joint_w"].T * inv_j[None, :]) / F, sh_j[None, :]])

    cvf = np.ascontiguousarray(p["cg_convf_w"].T)
    cvt = np.ascontiguousarray(p["cg_convt_w"].T)
    bfb = np.ascontiguousarray(p["cg_convf_b"].reshape(NBLK, CTX).T)
    btb = np.ascontiguousarray(p["cg_convt_b"].reshape(NBLK, CTX).T)

    sc = 1.0 / ((F + T) * TEMP)
    arw0 = np.concatenate([p["exp_res_w"], p["dep_res_w"], p["proj_res_w"]], 0).T * sc
    arb0 = np.concatenate([p["exp_res_b"], p["dep_res_b"], p["proj_res_b"]]) / TEMP
    arw = np.ascontiguousarray(np.vstack([arw0, arb0[None, :]]))

    drw_r = p["dr_w"].reshape(CEXP, 2 * M, CTX).transpose(1, 0, 2)
    drw0 = drw_r.reshape(2 * M * CEXP, CTX).T / (F + T)
    drb_r = p["dr_b"].reshape(CEXP, 2 * M).T.reshape(-1)
    drw = np.ascontiguousarray(np.vstack([drw0, drb_r[None, :]]))

    inv_e = p["exp_bn_gamma"] / np.sqrt(p["exp_bn_var"] + EPS)
    sh_e = p["exp_bn_beta"] - p["exp_bn_mean"] * inv_e
    ew = (p["exp_weight"] * inv_e[None, :, None]).transpose(0, 2, 1)  # [K,80,480]
    expw = np.ascontiguousarray(ew.transpose(1, 0, 2).reshape(CIN, K * CEXP))
    ebrow = sh_e[None, :]

    inv_d = p["dep_bn_gamma"] / np.sqrt(p["dep_bn_var"] + EPS)
    sh_d = p["dep_bn_beta"] - p["dep_bn_mean"] * inv_d
    dw = (p["dep_weight"] * inv_d[None, :, None, None]).reshape(K, NBLK, CTX, 9)
    slots12 = np.zeros((K, NBLK, CTX, 12), np.float32)
    perm = [0, 3, 1, 4, 2, 5, 6, 7, 8]  # tap hw-index per non-zero slot
    dstc = [0, 1, 2, 3, 4, 5, 6, 8, 10]
    for d, src in zip(dstc, perm):
        slots12[..., d] = dw[..., src]
    depw = np.ascontiguousarray(
        slots12.transpose(2, 0, 1, 3).reshape(CTX, K * NBLK * 12))
    bnbd = np.ascontiguousarray(sh_d.reshape(NBLK, CTX).T)

    inv_p = p["proj_bn_gamma"] / np.sqrt(p["proj_bn_var"] + EPS)
    sh_p = p["proj_bn_beta"] - p["proj_bn_mean"] * inv_p
    pw = p["proj_weight"] * inv_p[None, :, None]        # [K, 80, 480]
    pw_b = pw.reshape(K, COUT, NBLK, CTX).transpose(3, 0, 2, 1)
    projw = np.ascontiguousarray(pw_b.reshape(CTX, K * NBLK * COUT))
    pbias = sh_p[:, None]

    return dict(
        jlf=jlf.astype(bf), jlt=jlt.astype(bf),
        cvf=cvf.astype(bf), cvt=cvt.astype(bf), bfb=bfb, btb=btb,
        arw=arw.astype(bf), drw=drw.astype(bf),
        expw=expw.astype(bf), ebrow=np.ascontiguousarray(ebrow).astype(bf),
        depw=depw.astype(bf), bnbd=bnbd,
        projw=projw.astype(bf), pbias=pbias,
        identf=np.eye(128, dtype=np.float32),
        vtag=np.zeros((1, _VTAG), np.float32),
        identb=np.eye(128).astype(bf),
        onesbf=np.ones((1, FT), np.float32).astype(bf))


_BUILT = {}


def _build():
    if "nc" in _BUILT:
        return _BUILT["nc"]
    nc = bacc.Bacc("TRN2", target_bir_lowering=False, debug=False,
                   num_devices=NCORES)
    d = lambda n, s: nc.dram_tensor(n, list(s), F32, kind="ExternalInput").ap()
    b = lambda n, s: nc.dram_tensor(n, list(s), BF16, kind="ExternalInput").ap()
    io = [
        d("xs", (S, CIN, FT)),
        b("jlf", (CIN + 1, CTX)), b("jlt", (CIN + 1, CTX)),
        b("cvf", (CTX, CEXP)), b("cvt", (CTX, CEXP)),
        d("bfb", (CTX, NBLK)), d("btb", (CTX, NBLK)),
        b("arw", (CTX + 1, 3 * K)),
        b("drw", (CTX + 1, 2 * M * CEXP)),
        b("expw", (CIN, K * CEXP)), b("ebrow", (1, CEXP)),
        b("depw", (CTX, K * NBLK * 12)), d("bnbd", (CTX, NBLK)),
        b("projw", (CTX, K * NBLK * COUT)), d("pbias", (COUT, 1)),
        d("identf", (128, 128)), d("vtag", (1, _VTAG)),
        b("identb", (128, 128)),
        b("onesbf", (1, FT)),
        nc.dram_tensor("y", [S, COUT, FT], F32, kind="ExternalOutput").ap(),
    ]
    from contextlib import ExitStack
    with tile.TileContext(nc) as tc:
        with ExitStack() as es:
            with nc.allow_low_precision(
                    reason="bf16 reduction trees; validated vs f64 reference"):
                _emit(tc, io, es)
    nc.compile()
    _BUILT["nc"] = nc
    return nc


def _purge_stale_neff_cache():
    import shutil
    base = os.path.expanduser("~/.neuron-compile-cache")
    tag = os.path.join(base, f".dyblock_vtag_{_VTAG}")
    if os.path.exists(base) and not os.path.exists(tag):
        shutil.rmtree(base, ignore_errors=True)
        os.makedirs(base, exist_ok=True)
        open(tag, "w").close()


def kernel(**inputs):
    _purge_stale_neff_cache()
    nc = _build()
    host = _host_prep(inputs)
    x = np.ascontiguousarray(np.asarray(inputs["x"], dtype=np.float32)
                             .reshape(B, CIN, FT))
    in_maps = []
    for c in range(NCORES):
        m = {"xs": x[c * S:(c + 1) * S]}
        m.update(host)
        in_maps.append(m)
    res = run_bass_kernel_spmd(nc, in_maps, list(range(NCORES)))
    out = np.concatenate([res.results[c]["y"] for c in range(NCORES)], axis=0)
    return np.ascontiguousarray(out.reshape(B, COUT, F, T))


if __name__ == "__main__":
    import pickle
    with open('/tmp/ref_in.pkl', 'rb') as f:
        inp = pickle.load(f)
    got = kernel(**inp)
    exp = np.load('/tmp/ref_out.npy')
    rel = np.abs(got - exp).max() / np.abs(exp).max()
    print("rel err:", rel)


# revision 25
# speedup vs baseline: 1.0377x; 1.0377x over previous
"""Trainium2 Bass kernel for DY_Block (EfficientAT DyMN dynamic block).

Data-parallel over batch B=32 across 8 NeuronCores (4 samples/core).

Key techniques vs the naive formulation:
  - x loaded ONCE per sample via a gpsimd casting DMA (f32 DRAM -> bf16 SBUF);
    output stored via casting DMA (bf16 SBUF -> f32 DRAM).
  - All weight matmuls in bf16; biases ride matmuls as extra contraction rows
    (ones-row in the rhs), which is free since PE cost depends only on the
    output free size.
  - Expansion BN+hswish fused into ONE custom-DVE op (HSWISH_PSUM_ANT)
    that reads PSUM and writes the zero-padded fp8 depthwise input layout.
  - Depthwise 3x3 as fp8 DoubleRow matmuls: per-channel weights on diagonal
    lhsT tiles, taps paired along df (pair stride 128, ISA-aligned); odd
    dt-shifts are absorbed by shifting the PSUM write window instead of the
    (2B-aligned) read window; 512-col matmul halves respect the one-bank
    PSUM output limit while DVE/ACT epilogues read the full 1024-col tile.
  - DyReLU as y1/y2 on ACT (per-channel scale+bias cols) + max on DVE (2x),
    CoordAtt gating split as sigma_f on GPSIMD + sigma_t on DVE (2x).
  - Projection accumulates the residual via an identity matmul on bf16 x;
    output bias rides the ACT eviction.
"""
import dataclasses
import os
import zlib

import ml_dtypes
import numpy as np

import concourse.bass as bass
import concourse.bacc as bacc
import concourse.tile as tile
from concourse import mybir
from concourse.bass_utils import run_bass_kernel_spmd

import concourse.dve_ops as _dvo
from concourse.dve_spec import (
    Spec, Src0, C0, C1, C2, One, relu, minn,
    lower as _dve_lower, _has_src1,
)
from concourse.dve_uop import DveOpSpec

F32 = mybir.dt.float32
BF16 = mybir.dt.bfloat16
FP8 = mybir.dt.float8e4
AX = mybir.AxisListType
OP = mybir.AluOpType
AF = mybir.ActivationFunctionType
DRM = mybir.MatmulPerfMode.DoubleRow

B, CIN, CEXP, COUT, F, T = 32, 80, 480, 80, 32, 125
CTX, K, M = 120, 4, 2
TEMP = 30.0
EPS = 1e-3
NCORES = 8
S = B // NCORES
NBLK = CEXP // CTX
FT = F * T               # 4000
TP = 128                 # padded row stride in the fp8 depthwise layout
NR = 35                  # padded rows: f=-1..32 -> rows 0..33; row 34 zero
TOFF = 2                 # t offset inside a padded row
NCH = 4                  # 1000-col chunks per [*, 4000] plane
CHW = FT // NCH          # 1000
GFR = 8                  # f-rows per dep psum tile ([*, 8*128=1024] = 2 banks)

_VTAG = (zlib.crc32(open(__file__, 'rb').read()) % 997) + 2


def _register_dve_op(name, spec):
    for o in _dvo.OPS:
        if o.name == name:
            return o
    opcode = _dvo._CUSTOM_DVE_ROW_BASE + len(_dvo.OPS)
    shas = {}
    for ver in ("v3", "v4"):
        try:
            u = _dve_lower(spec, ver=ver)
            shas[ver] = DveOpSpec(
                name=name, opcode=opcode, uops=u, rd1_en=_has_src1(spec)
            ).sha(ver)
        except Exception:
            pass
    op = _dvo.DveOp(name, spec, subdim=False, uops_sha=shas)
    _dvo.OPS.append(op)
    _dvo._SUB_OPCODE_FOR_NAME[name] = opcode
    _dvo.CUSTOM_DVE_SPECS[name] = spec
    return op


# out = (psum + s1) * clip(psum*imm2 + s0, 0, 1); with imm2=1/6, s0=0.5,
# s1=0 this is exactly hswish(psum) (bias pre-added into psum by a ones-row).
HSWISH_PSUM = _register_dve_op(
    "HSWISH_PSUM_ANT",
    Spec(
        body=(Src0 + C1) * minn(relu(Src0 * C2 + C0), One),
        reference=lambda in0, in1, s0, s1, imm2: (
            (in0.astype(np.float32) + s1)
            * np.minimum(np.maximum(in0.astype(np.float32) * imm2 + s0, 0.0), 1.0)
        ),
    ),
)

# 12 lhsT diag slots: tap order; None = zero slot (DR pair filler)
_SLOTS = [(-1, -1), (0, -1), (-1, 0), (0, 0), (-1, 1), (0, 1),
          (1, -1), None, (1, 0), None, (1, 1), None]


def _ap_with(v, dims, offset):
    return dataclasses.replace(
        v, ap=type(v.ap)([list(v.ap[0])] + dims), offset=offset)


def _emit(tc, io, ctx):
    nc = tc.nc

    (xs, jlf, jlt, cvf, cvt, bfb, btb, arw, drw, expw, ebrow,
     depw, bnbd, projw, pbias, identf, vtag, identb, onesbf, yout) = io

    wpool = ctx.enter_context(tc.tile_pool(name="weights", bufs=1))
    ctx_pool = ctx.enter_context(tc.tile_pool(name="ctx", bufs=1))
    pspool = ctx.enter_context(tc.tile_pool(name="ps", bufs=2, space="PSUM"))
    work = ctx.enter_context(tc.tile_pool(name="work", bufs=2))
    xpool = ctx.enter_context(tc.tile_pool(name="xpool", bufs=1))
    zpool = ctx.enter_context(tc.tile_pool(name="zpool", bufs=1))

    def wtile(ap, tag):
        t = wpool.tile(list(ap.shape), ap.dtype, tag=tag)
        nc.sync.dma_start(t[:], ap)
        return t

    w_jlf = wtile(jlf, "jlf")          # [81,120] bf16
    w_jlt = wtile(jlt, "jlt")          # [81,120] bf16
    w_cvf = wtile(cvf, "cvf")          # [120,480] bf16
    w_cvt = wtile(cvt, "cvt")          # [120,480] bf16
    w_bfb = wtile(bfb, "bfb")          # [120,4] f32
    w_btb = wtile(btb, "btb")          # [120,4] f32
    w_arw = wtile(arw, "arw")          # [121,12] bf16
    w_drw = wtile(drw, "drw")          # [121,1920] bf16
    w_expw = wtile(expw, "expw")       # [80,1920] bf16 (k-major)
    w_ebrow = wtile(ebrow, "ebrow")    # [1,480] bf16
    w_depw = wtile(depw, "depw")       # [120, K*48] bf16 (k-major, 12 slots/blk)
    w_bnbd = wtile(bnbd, "bnbd")       # [120,4] f32
    w_projw = wtile(projw, "projw")    # [120, K*320] bf16 (k-major)
    w_pbias = wtile(pbias, "pbias")    # [80,1] f32
    w_idf = wtile(identf, "idf")       # [128,128] f32
    w_idb = wtile(identb, "idb")       # [128,128] bf16

    # ---------- load x (cast f32->bf16), ones rows ----------
    x0bs = []
    for s in range(S):
        xb = xpool.tile([CIN + 1, FT], BF16, tag=f"x0b{s}")
        nc.gpsimd.dma_start(xb[0:CIN, :], xs[s])
        nc.sync.dma_start(xb[CIN:CIN + 1, :], onesbf[0:1, 0:FT])
        x0bs.append(xb)

    # ---------- Phase A : context ----------
    ga = ctx_pool.tile([CIN + 1, S * (F + T)], BF16)
    nc.sync.dma_start(ga[CIN:CIN + 1, :], onesbf[0:1, 0:S * (F + T)])
    for s in range(S):
        nc.vector.tensor_reduce(
            ga[0:CIN, s * F:(s + 1) * F],
            x0bs[s][0:CIN].rearrange("p (f t) -> p f t", t=T), AX.X, OP.add)
        h1 = work.tile([CIN, 2000], BF16, tag="cth1")
        nc.vector.tensor_add(h1[:], x0bs[s][0:CIN, 0:2000], x0bs[s][0:CIN, 2000:4000])
        h2 = work.tile([CIN, 1000], BF16, tag="cth2")
        nc.vector.tensor_add(h2[:], h1[:, 0:1000], h1[:, 1000:2000])
        h3 = work.tile([CIN, 500], BF16, tag="cth3")
        nc.vector.tensor_add(h3[:], h2[:, 0:500], h2[:, 500:1000])
        h4 = work.tile([CIN, 250], BF16, tag="cth4")
        nc.vector.tensor_add(h4[:], h3[:, 0:250], h3[:, 250:500])
        nc.vector.tensor_add(ga[0:CIN, S * F + s * T: S * F + (s + 1) * T],
                             h4[:, 0:125], h4[:, 125:250])

    # joint conv + BN + hswish: psum has bias via ones-row
    SF, ST = S * F, S * T
    ps1 = pspool.tile([128, 1024], F32, tag="pse")
    nc.tensor.matmul(ps1[0:CTX, 0:SF], w_jlf[:], ga[:, 0:SF], start=True, stop=True)
    nc.tensor.matmul(ps1[0:CTX, 512:512 + ST], w_jlt[:], ga[:, SF:],
                     start=True, stop=True)
    gc_t = ctx_pool.tile([CTX, S * (F + T)], BF16)
    nc.vector._custom_dve(HSWISH_PSUM, out=gc_t[:, 0:SF], in0=ps1[0:CTX, 0:SF],
                          s0=0.5, s1=0.0, imm2=1.0 / 6.0)
    nc.vector._custom_dve(HSWISH_PSUM, out=gc_t[:, SF:], in0=ps1[0:CTX, 512:512 + ST],
                          s0=0.5, s1=0.0, imm2=1.0 / 6.0)

    g_c = ctx_pool.tile([CTX + 1, S], F32)
    tmp_r = ctx_pool.tile([CTX, S], F32)
    nc.vector.tensor_reduce(
        g_c[0:CTX, :], gc_t[:, 0:SF].rearrange("p (s f) -> p s f", s=S),
        AX.X, OP.add)
    nc.vector.tensor_reduce(
        tmp_r[:], gc_t[:, SF:].rearrange("p (s t) -> p s t", s=S), AX.X, OP.add)
    nc.vector.tensor_add(g_c[0:CTX, :], g_c[0:CTX, :], tmp_r[:])
    nc.gpsimd.dma_start(g_c[CTX:CTX + 1, :], onesbf[0:1, 0:S])  # cast bf16->f32
    g_cb = ctx_pool.tile([CTX + 1, S], BF16)
    nc.vector.tensor_copy(g_cb[:], g_c[:])

    # routing attention (3 heads x K experts)
    ps_a = pspool.tile([128, 1024], F32, tag="pse")
    nc.tensor.matmul(ps_a[0:S, 0:3 * K], g_cb[:], w_arw[:], start=True, stop=True)
    ex_t = ctx_pool.tile([S, 3 * K], F32)
    nc.scalar.activation(ex_t[:], ps_a[0:S, 0:3 * K], AF.Exp)
    s3 = ctx_pool.tile([S, 3], F32)
    nc.vector.tensor_reduce(
        s3[:], ex_t.rearrange("p (j k) -> p j k", j=3), AX.X, OP.add)
    rec3 = ctx_pool.tile([S, 3], F32)
    nc.vector.reciprocal(rec3[:], s3[:])
    attn = ctx_pool.tile([S, 3 * K], F32)
    for j in range(3):
        nc.vector.tensor_scalar(
            attn[:, j * K:(j + 1) * K], ex_t[:, j * K:(j + 1) * K],
            rec3[:, j:j + 1], None, OP.mult)
    att48 = ctx_pool.tile([S, 3 * K * S], F32)
    nc.vector.tensor_tensor(
        att48.rearrange("p (jk s) -> p jk s", s=S),
        attn.unsqueeze(2).broadcast_to((S, 3 * K, S)),
        w_idf[0:S, 0:S].unsqueeze(1).broadcast_to((S, 3 * K, S)),
        OP.mult)
    onesS = ctx_pool.tile([S, CTX], F32)
    nc.vector.memset(onesS[:], 1.0)
    ps_ab = pspool.tile([128, 1024], F32, tag="pse")
    nc.tensor.matmul(ps_ab[0:CTX, 0:3 * K * S], onesS[:], att48[:],
                     start=True, stop=True)
    att_b = ctx_pool.tile([CTX, 3 * K * S], F32)
    nc.scalar.copy(att_b[:], ps_ab[0:CTX, 0:3 * K * S])

    # DyReLU coefficients -> cj[j] [CTX, NBLK*S] (a1, a2, b1', b2')
    coefs = ctx_pool.tile([S, 2 * M * CEXP], F32)
    for j in range(4):
        ps_th = pspool.tile([128, 1024], F32, tag="pse")
        nc.tensor.matmul(ps_th[0:S, 0:CEXP], g_cb[:],
                         w_drw[:, j * CEXP:(j + 1) * CEXP], start=True, stop=True)
        nc.scalar.activation(coefs[:, j * CEXP:(j + 1) * CEXP],
                             ps_th[0:S, 0:CEXP], AF.Sigmoid)
    for j, (sc, of) in enumerate([(2.0, 0.0), (2.0, -1.0), (1.0, -0.5), (1.0, -0.5)]):
        nc.vector.tensor_scalar(coefs[:, j * CEXP:(j + 1) * CEXP],
                                coefs[:, j * CEXP:(j + 1) * CEXP],
                                sc, of, OP.mult, OP.add)
    cj = []
    for j in range(4):
        cj_t = ctx_pool.tile([CTX, NBLK * S], F32, tag=f"cj{j}")
        for blk in range(NBLK):
            ps_c = pspool.tile([128, 1024], F32, tag="pse")
            nc.tensor.transpose(
                ps_c[0:CTX, 0:S],
                coefs[:, j * CEXP + blk * CTX: j * CEXP + (blk + 1) * CTX],
                w_idf[0:S, 0:S])
            nc.scalar.copy(cj_t[:, blk * S:(blk + 1) * S], ps_c[0:CTX, 0:S])
        cj.append(cj_t)
    for i in range(2):  # fold dep-BN bias: b'_i = a_i*bnb + b_i
        for blk in range(NBLK):
            sl = slice(blk * S, (blk + 1) * S)
            nc.vector.scalar_tensor_tensor(
                cj[2 + i][:, sl], cj[i][:, sl], w_bnbd[:, blk:blk + 1],
                cj[2 + i][:, sl], OP.mult, OP.add)

    # CoordAtt gates (bf16)
    sigf = ctx_pool.tile([CTX, NBLK * S * F], BF16)
    sigt = ctx_pool.tile([CTX, NBLK * S * T], BF16)
    for blk in range(NBLK):
        ps_f = pspool.tile([128, 1024], F32, tag="pse")
        nc.tensor.matmul(ps_f[0:CTX, 0:SF], w_cvf[:, blk * CTX:(blk + 1) * CTX],
                         gc_t[:, 0:SF], start=True, stop=True)
        nc.scalar.activation(sigf[:, blk * SF:(blk + 1) * SF], ps_f[0:CTX, 0:SF],
                             AF.Sigmoid, bias=w_bfb[:, blk:blk + 1], scale=1.0)
        ps_t2 = pspool.tile([128, 1024], F32, tag="pse")
        nc.tensor.matmul(ps_t2[0:CTX, 0:ST], w_cvt[:, blk * CTX:(blk + 1) * CTX],
                         gc_t[:, SF:], start=True, stop=True)
        nc.scalar.activation(sigt[:, blk * ST:(blk + 1) * ST], ps_t2[0:CTX, 0:ST],
                             AF.Sigmoid, bias=w_btb[:, blk:blk + 1], scale=1.0)

    # ---------- Phase B : per-sample pipeline ----------
    sb_it = 0

    def make_mixes(s):
        def acol(jr, k):
            c0 = (jr * K + k) * S + s
            return att_b[:, c0:c0 + 1]

        # expert-weight mixes (tensor_scalar 4x + tensor_tensor 2x trees)
        def mix(dst, parts, src, w, jr, tg):
            t0 = work.tile([parts, w], BF16, tag=f"mx0{tg}", name=f"mx0{tg}{s}")
            t1 = work.tile([parts, w], BF16, tag=f"mx1{tg}", name=f"mx1{tg}{s}")
            nc.vector.tensor_scalar(t0[:], src(0), acol(jr, 0)[0:parts], None, OP.mult)
            nc.vector.tensor_scalar(t1[:], src(1), acol(jr, 1)[0:parts], None, OP.mult)
            nc.vector.tensor_tensor(t0[:], t0[:], t1[:], OP.add)
            nc.vector.tensor_scalar(t1[:], src(2), acol(jr, 2)[0:parts], None, OP.mult)
            nc.vector.tensor_tensor(t0[:], t0[:], t1[:], OP.add)
            nc.vector.tensor_scalar(t1[:], src(3), acol(jr, 3)[0:parts], None, OP.mult)
            nc.vector.tensor_tensor(dst, t0[:], t1[:], OP.add)

        web = work.tile([CIN + 1, CEXP], BF16, tag="web", name=f"web{s}")
        mix(web[0:CIN, :], CIN, lambda k: w_expw[:, k * CEXP:(k + 1) * CEXP],
            CEXP, 0, "e")
        nc.sync.dma_start(web[CIN:CIN + 1, :], ebrow)

        wd = work.tile([CTX, NBLK * 12], BF16, tag="wd", name=f"wd{s}")
        mix(wd[:], CTX, lambda k: w_depw[:, k * 48:(k + 1) * 48], NBLK * 12, 1, "d")

        wpb = work.tile([CTX, NBLK * COUT], BF16, tag="wpb", name=f"wpb{s}")
        mix(wpb[:], CTX, lambda k: w_projw[:, k * NBLK * COUT:(k + 1) * NBLK * COUT],
            NBLK * COUT, 2, "p")
        return web, wd, wpb

    mixes_next = make_mixes(0)
    for s in range(S):
        web, wd, wpb = mixes_next
        yo = work.tile([COUT, FT], F32, tag="yo")
        zs = [zpool.tile([CTX, FT], BF16, tag=f"z{blk}", bufs=2, name=f"z{blk}_{s}")
              for blk in range(NBLK)]

        # Software pipeline over the 16 (blk, G) units with a 2-stage lag so
        # each engine's in-order queue always has ready work:
        #   step i emits: matmuls(unit i) | y-stage(unit i-1) | gate(unit i-2)
        blk_state = {}

        def exp_chunk(blk, c, xe3):
            pse_t = pspool.tile([128, 1024], F32, tag="pse")
            lhs = web[:, blk * CTX:(blk + 1) * CTX]
            nc.tensor.matmul(pse_t[0:CTX, 0:512], lhs,
                             x0bs[s][:, c * CHW:c * CHW + 512],
                             start=True, stop=True)
            nc.tensor.matmul(pse_t[0:CTX, 512:1000], lhs,
                             x0bs[s][:, c * CHW + 512:(c + 1) * CHW],
                             start=True, stop=True)
            nc.vector._custom_dve(
                HSWISH_PSUM,
                out=xe3[:, 1 + GFR * c:1 + GFR * (c + 1), TOFF:TOFF + T],
                in0=pse_t[0:CTX, 0:CHW].rearrange("p (f t) -> p f t", t=T),
                s0=0.5, s1=0.0, imm2=1.0 / 6.0)

        def stage_exp(i):
            # prefetch: exp chunk (blk, c) lands 2 steps before dep unit
            # (blk, c-1) consumes it; exactly one chunk per step.
            nonlocal sb_it
            blk, c = divmod(i + 2, NCH)
            if blk >= NBLK:
                return
            if c == 0:
                dg = work.tile([CTX, 12 * TP], FP8, tag="dg")
                nc.vector.tensor_tensor(
                    dg[:].rearrange("p (i c) -> p i c", i=12),
                    w_idb[0:CTX, 0:TP].unsqueeze(1).broadcast_to((CTX, 12, TP)),
                    wd[:, blk * 12:(blk + 1) * 12]
                    .unsqueeze(2).broadcast_to((CTX, 12, TP)),
                    OP.mult)
                xe = work.tile([CTX, NR * TP], FP8, tag="xe")
                xe3 = xe[:].rearrange("p (f t) -> p f t", t=TP)
                if sb_it < 2:
                    nc.vector.memset(xe[:, 0:TP], 0.0)
                    nc.vector.memset(xe[:, 33 * TP:NR * TP], 0.0)
                    nc.vector.memset(xe3[:, 1:33, 0:TOFF], 0.0)
                    nc.vector.memset(xe3[:, 1:33, TOFF + T:TP], 0.0)
                sb_it += 1
                blk_state[blk] = (dg, xe, xe3)
            exp_chunk(blk, c, blk_state[blk][2])

        def stage_mm(i):
            blk, G = divmod(i, NCH)
            dg, xe, xe3 = blk_state[blk]
            psd_t = pspool.tile([128, 1024], F32, tag="psd")
            f0 = GFR * G
            for h in range(2):
                CB0 = (f0 + 4 * h) * TP
                CB2 = (f0 + 4 * h + 2) * TP
                plan = [(0, CB0, 1, 510), (1, CB0, 0, 512), (2, CB0 + 2, 1, 510),
                        (3, CB2, 1, 510), (4, CB2, 0, 512), (5, CB2 + 2, 1, 510)]
                for k, (p, W, o, L) in enumerate(plan):
                    nc.tensor.matmul(
                        psd_t[0:CTX, 512 * h + o: 512 * h + o + L],
                        _ap_with(dg[:], [[TP, 2], [1, CTX]], 2 * p * TP),
                        _ap_with(xe[:], [[TP, 2], [1, L]], W),
                        start=(k == 0), stop=(k == 5),
                        perf_mode=DRM, skip_group_check=True)
            return psd_t

        def stage_y(i, psd_t):
            blk, G = divmod(i, NCH)
            d3 = psd_t[0:CTX].rearrange("p (f t) -> p f t", t=TP)[:, 0:GFR,
                                                                 TOFF:TOFF + T]
            col = blk * S + s
            y1 = work.tile([CTX, CHW], BF16, tag="y1")
            nc.scalar.activation(y1[:].rearrange("p (f t) -> p f t", t=T), d3,
                                 AF.Identity, bias=cj[2][:, col:col + 1],
                                 scale=cj[0][:, col:col + 1])
            y2 = work.tile([CTX, CHW], BF16, tag="y2")
            nc.scalar.activation(y2[:].rearrange("p (f t) -> p f t", t=T), d3,
                                 AF.Identity, bias=cj[3][:, col:col + 1],
                                 scale=cj[1][:, col:col + 1])
            m = work.tile([CTX, CHW], BF16, tag="m", bufs=2)
            nc.vector.tensor_tensor(m[:], y1[:], y2[:], OP.max)
            return m

        def stage_gate_f(i, m):
            blk, G = divmod(i, NCH)
            col = blk * S + s
            m3 = m[:].rearrange("p (f t) -> p f t", t=T)
            gf_v = sigf[:, col * F + GFR * G: col * F + GFR * (G + 1)] \
                .unsqueeze(2).broadcast_to((CTX, GFR, T))
            nc.gpsimd.tensor_tensor(m3, m3, gf_v, OP.mult)

        def stage_gate_t(i, m):
            blk, G = divmod(i, NCH)
            col = blk * S + s
            m3 = m[:].rearrange("p (f t) -> p f t", t=T)
            gt_v = sigt[:, col * T:(col + 1) * T] \
                .unsqueeze(1).broadcast_to((CTX, GFR, T))
            nc.vector.tensor_tensor(
                zs[blk][:, G * CHW:(G + 1) * CHW].rearrange("p (f t) -> p f t", t=T),
                m3, gt_v, OP.mult)

        def proj_chunk(c):
            psp = pspool.tile([128, 1024], F32, tag="pse")
            for lo, ln in ((0, 512), (512, 488)):
                for blk in range(NBLK):
                    nc.tensor.matmul(
                        psp[0:COUT, lo:lo + ln],
                        wpb[:, blk * COUT:(blk + 1) * COUT],
                        zs[blk][:, c * CHW + lo:c * CHW + lo + ln],
                        start=(blk == 0), stop=False, skip_group_check=True)
                nc.tensor.matmul(
                    psp[0:COUT, lo:lo + ln], w_idb[0:CIN, 0:COUT],
                    x0bs[s][0:CIN, c * CHW + lo:c * CHW + lo + ln],
                    start=False, stop=True, skip_group_check=True)
            nc.scalar.activation(yo[:, c * CHW:(c + 1) * CHW], psp[0:COUT, 0:CHW],
                                 AF.Identity, bias=w_pbias[:, 0:1], scale=1.0)

        # step i emits: exp(i+2) | matmuls(i) | y-stage(i-1) | gates(i-2)
        NU = NBLK * NCH
        fifo = {}
        for i in range(-2, NU + 2):
            stage_exp(i)
            if 0 <= i < NU:
                fifo[i] = (stage_mm(i),)
            if 0 <= i - 1 < NU:
                fifo[i - 1] = (stage_y(i - 1, fifo[i - 1][0]),)
            if 0 <= i - 2 < NU:
                stage_gate_f(i - 2, fifo[i - 2][0])
                stage_gate_t(i - 2, fifo.pop(i - 2)[0])
                # proj chunk c only needs z cols c*CHW of every block; the
                # last writer of those is unit (3, c) = index 12+c.
                if i - 2 == NU - NCH and s + 1 < S:
                    mixes_next = make_mixes(s + 1)
                if i - 2 >= NU - NCH:
                    proj_chunk(i - 2 - (NU - NCH))
        nc.sync.dma_start(yout[s], yo[:])


def _host_prep(inputs):
    p = {k: np.asarray(v, dtype=np.float32) for k, v in inputs.items()}
    bf = ml_dtypes.bfloat16

    inv_j = p["cg_joint_gamma"] / np.sqrt(p["cg_joint_var"] + EPS)
    sh_j = p["cg_joint_beta"] - p["cg_joint_mean"] * inv_j
    jlf = np.vstack([(p["cg_joint_w"].T * inv_j[None, :]) / T, sh_j[None, :]])
    jlt = np.vstack([(p["cg_joint_w"].T * inv_j[None, :]) / F, sh_j[None, :]])

    cvf = np.ascontiguousarray(p["cg_convf_w"].T)
    cvt = np.ascontiguousarray(p["cg_convt_w"].T)
    bfb = np.ascontiguousarray(p["cg_convf_b"].reshape(NBLK, CTX).T)
    btb = np.ascontiguousarray(p["cg_convt_b"].reshape(NBLK, CTX).T)

    sc = 1.0 / ((F + T) * TEMP)
    arw0 = np.concatenate([p["exp_res_w"], p["dep_res_w"], p["proj_res_w"]], 0).T * sc
    arb0 = np.concatenate([p["exp_res_b"], p["dep_res_b"], p["proj_res_b"]]) / TEMP
    arw = np.ascontiguousarray(np.vstack([arw0, arb0[None, :]]))

    drw_r = p["dr_w"].reshape(CEXP, 2 * M, CTX).transpose(1, 0, 2)
    drw0 = drw_r.reshape(2 * M * CEXP, CTX).T / (F + T)
    drb_r = p["dr_b"].reshape(CEXP, 2 * M).T.reshape(-1)
    drw = np.ascontiguousarray(np.vstack([drw0, drb_r[None, :]]))

    inv_e = p["exp_bn_gamma"] / np.sqrt(p["exp_bn_var"] + EPS)
    sh_e = p["exp_bn_beta"] - p["exp_bn_mean"] * inv_e
    ew = (p["exp_weight"] * inv_e[None, :, None]).transpose(0, 2, 1)  # [K,80,480]
    expw = np.ascontiguousarray(ew.transpose(1, 0, 2).reshape(CIN, K * CEXP))
    ebrow = sh_e[None, :]

    inv_d = p["dep_bn_gamma"] / np.sqrt(p["dep_bn_var"] + EPS)
    sh_d = p["dep_bn_beta"] - p["dep_bn_mean"] * inv_d
    dw = (p["dep_weight"] * inv_d[None, :, None, None]).reshape(K, NBLK, CTX, 9)
    slots12 = np.zeros((K, NBLK, CTX, 12), np.float32)
    perm = [0, 3, 1, 4, 2, 5, 6, 7, 8]  # tap hw-index per non-zero slot
    dstc = [0, 1, 2, 3, 4, 5, 6, 8, 10]
    for d, src in zip(dstc, perm):
        slots12[..., d] = dw[..., src]
    depw = np.ascontiguousarray(
        slots12.transpose(2, 0, 1, 3).reshape(CTX, K * NBLK * 12))
    bnbd = np.ascontiguousarray(sh_d.reshape(NBLK, CTX).T)

    inv_p = p["proj_bn_gamma"] / np.sqrt(p["proj_bn_var"] + EPS)
    sh_p = p["proj_bn_beta"] - p["proj_bn_mean"] * inv_p
    pw = p["proj_weight"] * inv_p[None, :, None]        # [K, 80, 480]
    pw_b = pw.reshape(K, COUT, NBLK, CTX).transpose(3, 0, 2, 1)
    projw = np.ascontiguousarray(pw_b.reshape(CTX, K * NBLK * COUT))
    pbias = sh_p[:, None]

    return dict(
        jlf=jlf.astype(bf), jlt=jlt.astype(bf),
        cvf=cvf.astype(bf), cvt=cvt.astype(bf), bfb=bfb, btb=btb,
        arw=arw.astype(bf), drw=drw.astype(bf),
        expw=expw.astype(bf), ebrow=np.ascontiguousarray(ebrow).astype(bf),
        depw=depw.astype(bf), bnbd=bnbd,
        projw=projw.astype(bf), pbias=pbias,
        identf=np.eye(128, dtype=np.float32),
        vtag=np.zeros((1, _VTAG), np.float32),
        identb=np.eye(128).astype(bf),
        onesbf=np.ones((1, FT), np.float32).astype(bf))


_BUILT = {}


def _build():
    if "nc" in _BUILT:
        return _BUILT["nc"]
    nc = bacc.Bacc("TRN2", target_bir_lowering=False, debug=False,
                   num_devices=NCORES)
    d = lambda n, s: nc.dram_tensor(n, list(s), F32, kind="ExternalInput").ap()
    b = lambda n, s: nc.dram_tensor(n, list(s), BF16, kind="ExternalInput").ap()
    io = [
        d("xs", (S, CIN, FT)),
        b("jlf", (CIN + 1, CTX)), b("jlt", (CIN + 1, CTX)),
        b("cvf", (CTX, CEXP)), b("cvt", (CTX, CEXP)),
        d("bfb", (CTX, NBLK)), d("btb", (CTX, NBLK)),
        b("arw", (CTX + 1, 3 * K)),
        b("drw", (CTX + 1, 2 * M * CEXP)),
        b("expw", (CIN, K * CEXP)), b("ebrow", (1, CEXP)),
        b("depw", (CTX, K * NBLK * 12)), d("bnbd", (CTX, NBLK)),
        b("projw", (CTX, K * NBLK * COUT)), d("pbias", (COUT, 1)),
        d("identf", (128, 128)), d("vtag", (1, _VTAG)),
        b("identb", (128, 128)),
        b("onesbf", (1, FT)),
        nc.dram_tensor("y", [S, COUT, FT], F32, kind="ExternalOutput").ap(),
    ]
    from contextlib import ExitStack
    with tile.TileContext(nc) as tc:
        with ExitStack() as es:
            with nc.allow_low_precision(
                    reason="bf16 reduction trees; validated vs f64 reference"):
                _emit(tc, io, es)
    nc.compile()
    _BUILT["nc"] = nc
    return nc


def _purge_stale_neff_cache():
    import shutil
    base = os.path.expanduser("~/.neuron-compile-cache")
    tag = os.path.join(base, f".dyblock_vtag_{_VTAG}")
    if os.path.exists(base) and not os.path.exists(tag):
        shutil.rmtree(base, ignore_errors=True)
        os.makedirs(base, exist_ok=True)
        open(tag, "w").close()


def kernel(**inputs):
    _purge_stale_neff_cache()
    nc = _build()
    host = _host_prep(inputs)
    x = np.ascontiguousarray(np.asarray(inputs["x"], dtype=np.float32)
                             .reshape(B, CIN, FT))
    in_maps = []
    for c in range(NCORES):
        m = {"xs": x[c * S:(c + 1) * S]}
        m.update(host)
        in_maps.append(m)
    res = run_bass_kernel_spmd(nc, in_maps, list(range(NCORES)))
    out = np.concatenate([res.results[c]["y"] for c in range(NCORES)], axis=0)
    return np.ascontiguousarray(out.reshape(B, COUT, F, T))


if __name__ == "__main__":
    import pickle
    with open('/tmp/ref_in.pkl', 'rb') as f:
        inp = pickle.load(f)
    got = kernel(**inp)
    exp = np.load('/tmp/ref_out.npy')
    rel = np.abs(got - exp).max() / np.abs(exp).max()
    print("rel err:", rel)
